# revision 42
# baseline (speedup 1.0000x reference)
"""BRD4KANModel Trainium2 kernel, v6 (HW exec ~1.51 ms vs 2.37 ms baseline).

Data-parallel over batch across 8 NeuronCores (512 rows each, weights
replicated). Weights are preprocessed once on the host (static weight
layout prep): scaler and lambda^3 folded into the spline weights,
transposed to contraction-major [k, o], cast to bf16, packed per
(k-tile, o-block) so each weight DMA is one large contiguous transfer.
On-chip layout is feature-major (h^T: features on partitions, batch on
the free dim); no on-chip weight transposes, pure matmul streams on PE
(N=512, PE ~1.18 ms busy = the roofline for this bf16 FLOP count).

B-spline bases via the truncated-power identity: with y_m = relu(h-g_m)^3,
bases_ref[c] = lam^3 * (y_c - 4y_{c+1} + 6y_{c+2} - 4y_{c+3} + y_{c+4}).
HW quirks that shaped the elementwise pipeline:
 - DVE f32*f32 tensor multiply runs at 1/3 rate; f32*f16 with 16-bit
   out is full rate. So 7 of 10 cubes use ACT exp(3*ln(relu(.)+1e-20))
   and 3 use ACT square + DVE signed-cube STT + DVE max0 (balances ACT
   vs DVE). All ACT functions (relu/square/ln/exp/copy) live in ONE
   table set (natural_log_exp_and_others): sigmoid and silu are
   computed as exp/ln1p/exp chains to avoid ~2.7us table switches.
 - The 4th difference is one GpSimd TT add (y_c + y_{c+4}) plus three
   DVE scalar_tensor_tensor ops batched over all six c.
 - Engines execute their queues in order, so z work is emitted as
   fine-grained thunks drained between sweep o-blocks; otherwise PSUM
   evacs get buried behind a quarter's z ops and the PE stalls on PSUM
   ring reuse.
Layer sweeps: quarters 0,1 accumulate via SBUF hacc (fp16); quarters
2+3 are fused into one block-major PSUM-resident phase, which finalizes
the next layer's first h slices ~1.5 quarters early so cross-layer z
overlaps the tail sweep. xb / xf / h2 staging buffers alias bases/silu
pool slots (ring order proven safe) to fit the 6-pair bases ring in
SBUF.

This walrus build accepts only ONE semaphore wait per instruction;
_split_waits() post-processes the BIR JSON, hoisting excess waits onto
NoOps inserted just before each instruction on the same engine.
"""

import json
import os

import numpy as np

import concourse.bass as bass
import concourse.mybir as mybir
import concourse.tile as tile
from concourse.masks import make_identity

F32 = mybir.dt.float32
F16 = mybir.dt.float16
BF16 = mybir.dt.bfloat16
AF = mybir.ActivationFunctionType
OP = mybir.AluOpType

N_CORES = 8
BATCH = 4096
B = BATCH // N_CORES  # 512 per core
D = 2048
WIDTHS = [2048, 2048, 1024]
COEFF = 6
GRID_SIZE = 3
SPLINE_ORDER = 3
H = 2.0 / GRID_SIZE
GRID = [m * H - 1.0 - SPLINE_ORDER * H for m in range(GRID_SIZE + 2 * SPLINE_ORDER + 1)]
LAM3 = 1.0 / (6.0 * H**3)  # lambda^3, folded into spline weights on host

OBLK = 4            # o-tiles per PSUM block
KQ = 4              # k-tiles per quarter (fi=2048 -> 16 k-tiles -> 4 quarters)


def _split_waits(bir_bytes: bytes, keep: int = 1) -> bytes:
    d = json.loads(bir_bytes)
    for f in d["functions"]:
        for bb in f["blocks"]:
            new_insts = []
            for inst in bb["instructions"]:
                si = inst.get("sync_info")
                waits = (si or {}).get("on_wait") or []
                if len(waits) > keep:
                    extra = waits[:-keep]
                    inst["sync_info"]["on_wait"] = waits[-keep:]
                    for ci in range(0, len(extra), keep):
                        new_insts.append({
                            "name": f"{inst['name']}-w{ci}",
                            "opcode": "NoOp",
                            "engine": inst["engine"],
                            "ins": [],
                            "outs": [],
                            "debug": inst.get("debug"),
                            "sync_info": {"on_update": [],
                                          "on_wait": extra[ci:ci + keep]},
                        })
                new_insts.append(inst)
            bb["instructions"] = new_insts
    return json.dumps(d).encode()


def _patch_json(nc):
    orig = nc.to_json_bytes

    def patched():
        return _split_waits(orig())

    nc.to_json_bytes = patched
    return nc


def build():
    nc = bass.Bass()
    dims = [D] + WIDTHS
    x = nc.dram_tensor("x", [B, D], F32, kind="ExternalInput")
    wm = nc.dram_tensor("wm", [16, 8, 128, 512], BF16, kind="ExternalInput")
    mb = nc.dram_tensor("mb", [2 * D], F32, kind="ExternalInput")
    wk = []
    for l in range(3):
        nblk = dims[l + 1] // (OBLK * 128)
        wk.append(nc.dram_tensor(f"wk{l}", [16, nblk, 7, 128, OBLK * 128],
                                 BF16, kind="ExternalInput"))
    hw = nc.dram_tensor("hw", [WIDTHS[-1], 2], BF16, kind="ExternalInput")
    hb = nc.dram_tensor("hb", [2, 1], F32, kind="ExternalInput")
    out = nc.dram_tensor("out", [2, B], F32, kind="ExternalOutput")

    with tile.TileContext(nc) as tc:
        with tc.tile_pool(name="consts", bufs=1) as consts, \
             tc.tile_pool(name="hring", bufs=2) as hring, \
             tc.tile_pool(name="basesp", bufs=6) as basesp, \
             tc.tile_pool(name="yp", bufs=2) as yp, \
             tc.tile_pool(name="pp", bufs=2) as pp, \
             tc.tile_pool(name="rp", bufs=2) as rp, \
             tc.tile_pool(name="silup", bufs=3) as silup, \
             tc.tile_pool(name="wkpa", bufs=2) as wkpa, \
             tc.tile_pool(name="wkpb", bufs=2) as wkpb, \
             tc.tile_pool(name="wmp", bufs=4) as wmp, \
             tc.tile_pool(name="psA", bufs=7, space="PSUM") as psA, \
             tc.tile_pool(name="psT", bufs=1, space="PSUM") as psT:

            # ---- constants ----
            ident = consts.tile([128, 128], BF16, tag="ident")
            make_identity(nc, ident)
            mb_sb = consts.tile([128, 32], F32, tag="mb")
            nc.sync.dma_start(mb_sb, mb[:].rearrange("(t p) -> p t", p=128))
            mbneg = consts.tile([128, 32], F32, tag="mbneg")
            nc.vector.tensor_scalar(mbneg, mb_sb, -1.0, None, OP.mult)
            hw_sb = consts.tile([128, 8, 2], BF16, tag="hw")
            nc.sync.dma_start(hw_sb, hw[:].rearrange("(t p) c -> p t c", p=128))
            hb_sb = consts.tile([2, 1], F32, tag="hb")
            nc.sync.dma_start(hb_sb, hb[:])
            grid_sb = consts.tile([128, 10], F32, tag="grid")
            for m in range(10):
                nc.vector.memset(grid_sb[:, m:m + 1], float(-GRID[m]))
            tiny = consts.tile([128, 1], F32, tag="tiny")
            nc.vector.memset(tiny, 1e-20)

            # ---- x: load, cast bf16, PE-transpose to feature-major ----
            # xb tiles live inside two bases-pool slots (dead before the
            # bases ring wraps back to them)
            bd0 = basesp.tile([128, COEFF, 2, B], BF16, tag="bases",
                              name="bd0")
            bd1 = basesp.tile([128, COEFF, 2, B], BF16, tag="bases",
                              name="bd1")
            bd0f = bd0.rearrange("p c i b -> p (c i b)")
            bd1f = bd1.rearrange("p c i b -> p (c i b)")
            xb = [(bd0f if i < 12 else bd1f)[:, (i % 12) * B:(i % 12 + 1) * B]
                  for i in range(16)]
            for bt in range(4):
                xf = silup.tile([128, KQ, B], BF16, tag="silu",
                                name="xf").rearrange("p k b -> p (k b)")
                nc.gpsimd.dma_start(xf, x[bt * 128:(bt + 1) * 128, :])
                for g in range(4):
                    pt = psT.tile([128, 512], BF16, tag="pt")
                    for qq in range(4):
                        i = 4 * g + qq
                        nc.tensor.transpose(pt[:, qq * 128:(qq + 1) * 128],
                                            xf[:, i * 128:(i + 1) * 128], ident)
                    for qq in range(4):
                        i = 4 * g + qq
                        nc.vector.tensor_copy(xb[i][:, bt * 128:(bt + 1) * 128],
                                              pt[:, qq * 128:(qq + 1) * 128])

            # ---- multiplicative layer ----
            # h0 = sigmoid(x@Wg + bg) * (x@Wv + bv), feature-major fp16
            sig_t = hring.tile([128, 16, B], F16, tag="h", name="sig")
            h_in = hring.tile([128, 16, B], F16, tag="h", name="h0")

            def mult_block(blk, is_gate, j):
                accs = [psA.tile([128, B], F32, tag="acc", name=f"m{blk}_{o}")
                        for o in range(OBLK)]
                for kt in range(16):
                    ws = wmp.tile([128, 512], BF16, tag="wm")
                    eng = nc.sync if kt % 2 == 0 else nc.scalar
                    eng.dma_start(ws, wm[kt, blk])
                    for o in range(OBLK):
                        nc.tensor.matmul(accs[o], ws[:, o * 128:(o + 1) * 128],
                                         xb[kt], start=(kt == 0),
                                         stop=(kt == 15))
                for o in range(OBLK):
                    ot = 4 * j + o
                    if is_gate:
                        # sigmoid via exp/ln1p/exp: stays in the ln_exp
                        # ACT table set (no table switching)
                        e = rp.tile([128, B], F32, tag="r", name="sige")
                        nc.scalar.activation(e, accs[o], AF.Exp, scale=-1.0,
                                             bias=mbneg[:, ot:ot + 1])
                        nc.scalar.activation(e, e, AF.Ln, bias=1.0)
                        nc.scalar.activation(sig_t[:, ot, :], e, AF.Exp,
                                             scale=-1.0)
                    else:
                        nc.vector.scalar_tensor_tensor(
                            h_in[:, ot, :], accs[o], mb_sb[:, 16 + ot:17 + ot],
                            sig_t[:, ot, :], OP.add, OP.mult)

            for j in range(4):
                mult_block(j, True, j)        # gate o-tiles 4j..4j+3
            # val blocks emitted below (interleaved with layer-0 z work)

            # ---- KAN layers ----
            SQ_SET = (2, 5, 8)  # m's using square+signed-cube; rest use ln/exp

            from collections import deque
            pending = deque()

            def drain(n):
                for _ in range(n):
                    if not pending:
                        return
                    pending.popleft()()

            class ZQ:
                """Lazily-emitted bases+silu for k-tiles [4q,4q+4).
                Emission is split into 8 thunks (silu chain / z chunk per
                i-tile pair) so evac ops of concurrent sweeps are not
                buried behind a whole quarter of z work in the engine
                queues."""

                def __init__(self, l, q, h_t):
                    self.l, self.q, self.h_t = l, q, h_t
                    self.st = None
                    self.pairs = []
                    self.remaining = 2 * KQ

                    def wrap(fn, ki):
                        def run():
                            fn(ki)
                            self.remaining -= 1
                        return run

                    for ki in range(KQ):
                        pending.append(wrap(self._silu, ki))
                        pending.append(wrap(self._chunk, ki))

                def ensure(self):
                    while self.remaining > 0 and pending:
                        pending.popleft()()

                def _silu(self, ki):
                    if self.st is None:
                        self.st = silup.tile([128, KQ, B], BF16, tag="silu",
                                             name=f"si{self.l}_{self.q}")
                    it = 4 * self.q + ki
                    h_t = self.h_t
                    # silu(h) = h * sigmoid(h), sigmoid via exp/ln1p/exp
                    e = rp.tile([128, B], F32, tag="r", name="sile")
                    nc.scalar.activation(e, h_t[:, it, :], AF.Exp, scale=-1.0)
                    nc.scalar.activation(e, e, AF.Ln, bias=1.0)
                    nc.scalar.activation(e, e, AF.Exp, scale=-1.0)
                    nc.vector.scalar_tensor_tensor(
                        self.st[:, ki, :], e, 0.0, h_t[:, it, :],
                        OP.add, OP.mult)

                def _chunk(self, ki):
                    if ki % 2 == 0:
                        self.pairs.append(basesp.tile(
                            [128, COEFF, 2, B], BF16, tag="bases",
                            name=f"ba{self.l}_{self.q}_{ki}"))
                    bt6 = self.pairs[ki // 2]
                    irel = ki % 2
                    hsl = self.h_t[:, 4 * self.q + ki, :]  # [128,512] contig
                    y = yp.tile([128, 10, B], F32, tag="y")
                    yf = y.rearrange("p m b -> p (m b)")
                    for m in range(10):
                        if m in SQ_SET:
                            # (h-g)^2 -> signed cube -> clamp to >=0
                            nc.scalar.activation(y[:, m], hsl, AF.Square,
                                                 bias=grid_sb[:, m:m + 1])
                            nc.vector.scalar_tensor_tensor(
                                y[:, m], hsl, grid_sb[:, m:m + 1],
                                y[:, m], OP.add, OP.mult)
                            nc.vector.tensor_scalar(y[:, m], y[:, m], 0.0,
                                                    None, OP.max)
                        else:
                            # relu^3 = exp(3*ln(relu(h-g)+1e-20))
                            r = rp.tile([128, B], F32, tag="r")
                            nc.scalar.activation(r, hsl, AF.Relu,
                                                 bias=grid_sb[:, m:m + 1])
                            nc.scalar.activation(r, r, AF.Ln,
                                                 bias=tiny[:, 0:1])
                            nc.scalar.activation(y[:, m], r, AF.Exp,
                                                 scale=3.0)
                    a = pp.tile([128, COEFF, B], F32, tag="a")
                    af = a.rearrange("p c b -> p (c b)")
                    nc.gpsimd.tensor_tensor(af, yf[:, 0:3072],
                                            yf[:, 2048:5120], OP.add)
                    nc.vector.scalar_tensor_tensor(af, yf[:, 512:3584], -4.0,
                                                   af, OP.mult, OP.add)
                    nc.vector.scalar_tensor_tensor(af, yf[:, 1536:4608], -4.0,
                                                   af, OP.mult, OP.add)
                    nc.vector.scalar_tensor_tensor(
                        bt6[:, :, irel, :], y[:, 2:8], 6.0, a,
                        OP.mult, OP.add)

            def z_quarter(l, q, h_t):
                return ZQ(l, q, h_t)

            def sweep(l, kt_lo, kt_hi, hacc, h2_t, zq, mode, post_blk,
                      drain_n):
                """matmuls over k-tiles [kt_lo, kt_hi) for all o of layer l.
                mode: 'copy' (PSUM->hacc), 'add' (hacc += PSUM), or
                'final' (h_next = hacc + PSUM; bf16 h2 when l==2)."""
                for q in set(kt // 4 for kt in range(kt_lo, kt_hi)):
                    zq[q].ensure()
                fo = dims[l + 1]
                nblk = fo // (OBLK * 128)
                for blk in range(nblk):
                    accs = [psA.tile([128, B], F32, tag="acc",
                                     name=f"a{l}_{kt_lo}_{blk}_{o}")
                            for o in range(OBLK)]
                    for kt in range(kt_lo, kt_hi):
                        # layer-0 opening strips ride the idle SWDGE queue so
                        # they don't FIFO behind the mult layer's 128 strips
                        ea = eb = nc.gpsimd if (l == 0 and kt < 8) else None
                        wa = wkpa.tile([128, 4, OBLK * 128], BF16, tag="wka")
                        (ea or nc.sync).dma_start(
                            wa, wk[l][kt, blk, 0:4].rearrange("s p f -> p s f"))
                        wb = wkpb.tile([128, 3, OBLK * 128], BF16, tag="wkb")
                        (eb or nc.scalar).dma_start(
                            wb, wk[l][kt, blk, 4:7].rearrange("s p f -> p s f"))
                        z0 = zq[kt // 4]
                        for s in range(7):
                            wt_sl = wa[:, s, :] if s < 4 else wb[:, s - 4, :]
                            rhs = (z0.st[:, kt % 4, :] if s == 0
                                   else z0.pairs[(kt % 4) // 2][:, s - 1,
                                                                kt % 2, :])
                            for o in range(OBLK):
                                osl = slice(o * 128, (o + 1) * 128)
                                nc.tensor.matmul(
                                    accs[o], wt_sl[:, osl], rhs,
                                    start=(kt == kt_lo and s == 0),
                                    stop=(kt == kt_hi - 1 and s == 6))
                    for o in range(OBLK):
                        ot = OBLK * blk + o
                        if mode == "copy":
                            nc.scalar.copy(hacc[:, ot, :], accs[o])
                        elif mode == "final" and l == 2:
                            nc.vector.tensor_tensor(h2_t[:, ot, :], accs[o],
                                                    hacc[:, ot, :], OP.add)
                        else:
                            nc.vector.tensor_tensor(hacc[:, ot, :], accs[o],
                                                    hacc[:, ot, :], OP.add)
                    if post_blk is not None:
                        post_blk(blk)
                    drain(drain_n)

            # h2 lives in a bases-pool slot (allocated at layer-2 time)
            h2_holder = [None]

            # ---- layer flow ----
            # mult val blocks, with layer-0 z starting as soon as the h
            # slices each needs are final
            zq = {}
            for j in range(4):
                mult_block(4 + j, False, j)
                if j == 0:
                    zq[0] = z_quarter(0, 0, h_in)
                elif j == 1:
                    zq[1] = z_quarter(0, 1, h_in)
                drain(3 if j else 0)

            for l in range(3):
                nblk = dims[l + 1] // (OBLK * 128)
                dn = 8 // nblk
                hacc = hring.tile([128, 16, B], F16, tag="h", name=f"hacc{l}")
                zq[2] = z_quarter(l, 2, h_in)
                sweep(l, 0, 4, hacc, None, zq, "copy", None, dn + 1)
                zq[3] = z_quarter(l, 3, h_in)
                sweep(l, 4, 8, hacc, None, zq, "add", None, dn + 1)
                if l == 2:
                    bt = basesp.tile([128, COEFF, 2, B], BF16, tag="bases",
                                     name="h2t")
                    h2_holder[0] = bt.rearrange("p c i b -> p (c i) b")
                nxt_h = hacc
                nxt_zq = {}

                def post_blk(blk, l=l, nxt_h=nxt_h, nxt_zq=nxt_zq):
                    if l < 2 and blk <= 1:
                        nxt_zq[blk] = z_quarter(l + 1, blk, nxt_h)

                h2v = h2_holder[0]
                sweep(l, 8, 16, hacc, h2v, zq, "final", post_blk, dn + 1)
                h_in = hacc
                zq = nxt_zq

            # ---- heads ----
            h2_t = h2_holder[0]
            acc2 = psA.tile([128, B], F32, tag="acc", name="headacc")
            for kt in range(8):
                nc.tensor.matmul(acc2[0:2, :], hw_sb[:, kt, :],
                                 h2_t[:, kt, :], start=(kt == 0),
                                 stop=(kt == 7))
            res = consts.tile([2, B], F32, tag="res")
            nc.vector.tensor_scalar(res, acc2[0:2, :], hb_sb[:, 0:1], None,
                                    OP.add)
            nc.sync.dma_start(out[:], res)

    return _patch_json(nc)


_NC = None
_PACKED = None


def _pack_weights(inputs):
    import ml_dtypes
    bf16 = ml_dtypes.bfloat16
    dims = [D] + WIDTHS
    packed = {}
    # mult: wT[i, o] -> [16, 8, 128, 512]
    wT = np.ascontiguousarray(np.asarray(inputs["mult_w"], np.float32).T)
    packed["wm"] = np.ascontiguousarray(
        wT.reshape(16, 128, 8, 4, 128).transpose(0, 2, 1, 3, 4)
        .reshape(16, 8, 128, 512)).astype(bf16)
    packed["mb"] = np.ascontiguousarray(np.asarray(inputs["mult_b"], np.float32))
    for l in range(3):
        fi, fo = dims[l], dims[l + 1]
        nblk = fo // (OBLK * 128)
        bw = np.asarray(inputs[f"base_w{l}"], np.float32)
        sw = np.asarray(inputs[f"spline_w{l}"], np.float32)
        sc = np.asarray(inputs[f"scaler{l}"], np.float32)
        S = np.empty((fi, 7, fo), np.float32)
        S[:, 0, :] = bw.T
        S[:, 1:, :] = (sw * (sc[:, :, None] * LAM3)).transpose(1, 2, 0)
        packed[f"wk{l}"] = np.ascontiguousarray(
            S.reshape(16, 128, 7, nblk, OBLK * 128).transpose(0, 3, 2, 1, 4)
        ).astype(bf16)
    packed["hw"] = np.ascontiguousarray(np.stack(
        [np.asarray(inputs["reg_w"], np.float32)[0],
         np.asarray(inputs["aux_w"], np.float32)[0]], axis=1)).astype(bf16)
    packed["hb"] = np.array(
        [[float(np.asarray(inputs["reg_b"]).reshape(-1)[0])],
         [float(np.asarray(inputs["aux_b"]).reshape(-1)[0])]], np.float32)
    return packed


def kernel(**inputs):
    global _NC, _PACKED
    from concourse.bass_utils import run_bass_kernel_spmd

    if _NC is None:
        _NC = build()
    _PACKED = _pack_weights(inputs)
    x_full = np.ascontiguousarray(np.asarray(inputs["x"], np.float32))
    per_core = []
    for c in range(N_CORES):
        m = dict(_PACKED)
        m["x"] = np.ascontiguousarray(x_full[c * B:(c + 1) * B])
        per_core.append(m)
    res = run_bass_kernel_spmd(_NC, per_core, core_ids=list(range(N_CORES)))
    reg = np.concatenate([res.results[c]["out"][0] for c in range(N_CORES)])
    aux = np.concatenate([res.results[c]["out"][1] for c in range(N_CORES)])
    kernel.last_results = res
    return reg, aux


# revision 43
# speedup vs baseline: 1.0194x; 1.0194x over previous
"""BRD4KANModel Trainium2 kernel, v6 (HW exec ~1.51 ms vs 2.37 ms baseline).

Data-parallel over batch across 8 NeuronCores (512 rows each, weights
replicated). Weights are preprocessed once on the host (static weight
layout prep): scaler and lambda^3 folded into the spline weights,
transposed to contraction-major [k, o], cast to bf16, packed per
(k-tile, o-block) so each weight DMA is one large contiguous transfer.
On-chip layout is feature-major (h^T: features on partitions, batch on
the free dim); no on-chip weight transposes, pure matmul streams on PE
(N=512, PE ~1.18 ms busy = the roofline for this bf16 FLOP count).

B-spline bases via the truncated-power identity: with y_m = relu(h-g_m)^3,
bases_ref[c] = lam^3 * (y_c - 4y_{c+1} + 6y_{c+2} - 4y_{c+3} + y_{c+4}).
HW quirks that shaped the elementwise pipeline:
 - DVE f32*f32 tensor multiply runs at 1/3 rate; f32*f16 with 16-bit
   out is full rate. So 7 of 10 cubes use ACT exp(3*ln(relu(.)+1e-20))
   and 3 use ACT square + DVE signed-cube STT + DVE max0 (balances ACT
   vs DVE). All ACT functions (relu/square/ln/exp/copy) live in ONE
   table set (natural_log_exp_and_others): sigmoid and silu are
   computed as exp/ln1p/exp chains to avoid ~2.7us table switches.
 - The 4th difference is one GpSimd TT add (y_c + y_{c+4}) plus three
   DVE scalar_tensor_tensor ops batched over all six c.
 - Engines execute their queues in order, so z work is emitted as
   fine-grained thunks drained between sweep o-blocks; otherwise PSUM
   evacs get buried behind a quarter's z ops and the PE stalls on PSUM
   ring reuse.
Layer sweeps: quarters 0,1 accumulate via SBUF hacc (fp16); quarters
2+3 are fused into one block-major PSUM-resident phase, which finalizes
the next layer's first h slices ~1.5 quarters early so cross-layer z
overlaps the tail sweep. xb / xf / h2 staging buffers alias bases/silu
pool slots (ring order proven safe) to fit the 6-pair bases ring in
SBUF.

This walrus build accepts only ONE semaphore wait per instruction;
_split_waits() post-processes the BIR JSON, hoisting excess waits onto
NoOps inserted just before each instruction on the same engine.
"""

import json
import os

import numpy as np

import concourse.bass as bass
import concourse.mybir as mybir
import concourse.tile as tile
from concourse.masks import make_identity

F32 = mybir.dt.float32
F16 = mybir.dt.float16
BF16 = mybir.dt.bfloat16
AF = mybir.ActivationFunctionType
OP = mybir.AluOpType

N_CORES = 8
BATCH = 4096
B = BATCH // N_CORES  # 512 per core
D = 2048
WIDTHS = [2048, 2048, 1024]
COEFF = 6
GRID_SIZE = 3
SPLINE_ORDER = 3
H = 2.0 / GRID_SIZE
GRID = [m * H - 1.0 - SPLINE_ORDER * H for m in range(GRID_SIZE + 2 * SPLINE_ORDER + 1)]
LAM3 = 1.0 / (6.0 * H**3)  # lambda^3, folded into spline weights on host

OBLK = 4            # o-tiles per PSUM block
KQ = 4              # k-tiles per quarter (fi=2048 -> 16 k-tiles -> 4 quarters)


def _split_waits(bir_bytes: bytes, keep: int = 1) -> bytes:
    d = json.loads(bir_bytes)
    for f in d["functions"]:
        for bb in f["blocks"]:
            new_insts = []
            for inst in bb["instructions"]:
                si = inst.get("sync_info")
                waits = (si or {}).get("on_wait") or []
                if len(waits) > keep:
                    extra = waits[:-keep]
                    inst["sync_info"]["on_wait"] = waits[-keep:]
                    for ci in range(0, len(extra), keep):
                        new_insts.append({
                            "name": f"{inst['name']}-w{ci}",
                            "opcode": "NoOp",
                            "engine": inst["engine"],
                            "ins": [],
                            "outs": [],
                            "debug": inst.get("debug"),
                            "sync_info": {"on_update": [],
                                          "on_wait": extra[ci:ci + keep]},
                        })
                new_insts.append(inst)
            bb["instructions"] = new_insts
    return json.dumps(d).encode()


def _patch_json(nc):
    orig = nc.to_json_bytes

    def patched():
        return _split_waits(orig())

    nc.to_json_bytes = patched
    return nc


def build():
    nc = bass.Bass()
    dims = [D] + WIDTHS
    x = nc.dram_tensor("x", [B, D], F32, kind="ExternalInput")
    wm = nc.dram_tensor("wm", [16, 8, 128, 512], BF16, kind="ExternalInput")
    mb = nc.dram_tensor("mb", [2 * D], F32, kind="ExternalInput")
    wk = []
    for l in range(3):
        nblk = dims[l + 1] // (OBLK * 128)
        wk.append(nc.dram_tensor(f"wk{l}", [16, nblk, 7, 128, OBLK * 128],
                                 BF16, kind="ExternalInput"))
    hw = nc.dram_tensor("hw", [WIDTHS[-1], 2], BF16, kind="ExternalInput")
    hb = nc.dram_tensor("hb", [2, 1], F32, kind="ExternalInput")
    out = nc.dram_tensor("out", [2, B], F32, kind="ExternalOutput")

    with tile.TileContext(nc) as tc:
        with tc.tile_pool(name="consts", bufs=1) as consts, \
             tc.tile_pool(name="hring", bufs=2) as hring, \
             tc.tile_pool(name="basesp", bufs=6) as basesp, \
             tc.tile_pool(name="yp", bufs=2) as yp, \
             tc.tile_pool(name="pp", bufs=2) as pp, \
             tc.tile_pool(name="rp", bufs=2) as rp, \
             tc.tile_pool(name="silup", bufs=3) as silup, \
             tc.tile_pool(name="wkpa", bufs=2) as wkpa, \
             tc.tile_pool(name="wkpb", bufs=2) as wkpb, \
             tc.tile_pool(name="wmp", bufs=4) as wmp, \
             tc.tile_pool(name="psA", bufs=7, space="PSUM") as psA, \
             tc.tile_pool(name="psT", bufs=1, space="PSUM") as psT:

            # ---- constants ----
            ident = consts.tile([128, 128], BF16, tag="ident")
            make_identity(nc, ident)
            mb_sb = consts.tile([128, 32], F32, tag="mb")
            nc.sync.dma_start(mb_sb, mb[:].rearrange("(t p) -> p t", p=128))
            mbneg = consts.tile([128, 32], F32, tag="mbneg")
            nc.vector.tensor_scalar(mbneg, mb_sb, -1.0, None, OP.mult)
            hw_sb = consts.tile([128, 8, 2], BF16, tag="hw")
            nc.sync.dma_start(hw_sb, hw[:].rearrange("(t p) c -> p t c", p=128))
            hb_sb = consts.tile([2, 1], F32, tag="hb")
            nc.sync.dma_start(hb_sb, hb[:])
            grid_sb = consts.tile([128, 10], F32, tag="grid")
            for m in range(10):
                nc.vector.memset(grid_sb[:, m:m + 1], float(-GRID[m]))
            tiny = consts.tile([128, 1], F32, tag="tiny")
            nc.vector.memset(tiny, 1e-20)

            # ---- x: load, cast bf16, PE-transpose to feature-major ----
            # xb tiles live inside two bases-pool slots (dead before the
            # bases ring wraps back to them)
            bd0 = basesp.tile([128, COEFF, 2, B], BF16, tag="bases",
                              name="bd0")
            bd1 = basesp.tile([128, COEFF, 2, B], BF16, tag="bases",
                              name="bd1")
            bd0f = bd0.rearrange("p c i b -> p (c i b)")
            bd1f = bd1.rearrange("p c i b -> p (c i b)")
            xb = [(bd0f if i < 12 else bd1f)[:, (i % 12) * B:(i % 12 + 1) * B]
                  for i in range(16)]
            for bt in range(4):
                xf = silup.tile([128, KQ, B], BF16, tag="silu",
                                name="xf").rearrange("p k b -> p (k b)")
                nc.gpsimd.dma_start(xf, x[bt * 128:(bt + 1) * 128, :])
                for g in range(4):
                    pt = psT.tile([128, 512], BF16, tag="pt")
                    for qq in range(4):
                        i = 4 * g + qq
                        nc.tensor.transpose(pt[:, qq * 128:(qq + 1) * 128],
                                            xf[:, i * 128:(i + 1) * 128], ident)
                    for qq in range(4):
                        i = 4 * g + qq
                        nc.vector.tensor_copy(xb[i][:, bt * 128:(bt + 1) * 128],
                                              pt[:, qq * 128:(qq + 1) * 128])

            # ---- multiplicative layer ----
            # h0 = sigmoid(x@Wg + bg) * (x@Wv + bv), feature-major fp16
            sig_t = hring.tile([128, 16, B], F16, tag="h", name="sig")
            h_in = hring.tile([128, 16, B], F16, tag="h", name="h0")

            def mult_block(blk, is_gate, j):
                accs = [psA.tile([128, B], F32, tag="acc", name=f"m{blk}_{o}")
                        for o in range(OBLK)]
                for kt in range(16):
                    ws = wmp.tile([128, 512], BF16, tag="wm")
                    eng = nc.sync if kt % 2 == 0 else nc.scalar
                    eng.dma_start(ws, wm[kt, blk])
                    for o in range(OBLK):
                        nc.tensor.matmul(accs[o], ws[:, o * 128:(o + 1) * 128],
                                         xb[kt], start=(kt == 0),
                                         stop=(kt == 15))
                for o in range(OBLK):
                    ot = 4 * j + o
                    if is_gate:
                        # sigmoid via exp/ln1p/exp: stays in the ln_exp
                        # ACT table set (no table switching)
                        e = rp.tile([128, B], F32, tag="r", name="sige")
                        nc.scalar.activation(e, accs[o], AF.Exp, scale=-1.0,
                                             bias=mbneg[:, ot:ot + 1])
                        nc.scalar.activation(e, e, AF.Ln, bias=1.0)
                        nc.scalar.activation(sig_t[:, ot, :], e, AF.Exp,
                                             scale=-1.0)
                    else:
                        nc.vector.scalar_tensor_tensor(
                            h_in[:, ot, :], accs[o], mb_sb[:, 16 + ot:17 + ot],
                            sig_t[:, ot, :], OP.add, OP.mult)

            for j in range(4):
                mult_block(j, True, j)        # gate o-tiles 4j..4j+3
            # val blocks emitted below (interleaved with layer-0 z work)

            # ---- KAN layers ----
            SQ_SET = (2, 5, 8)  # m's using square+signed-cube; rest use ln/exp

            from collections import deque
            pending = deque()

            def drain(n):
                for _ in range(n):
                    if not pending:
                        return
                    pending.popleft()()

            class ZQ:
                """Lazily-emitted bases+silu for k-tiles [4q,4q+4).
                Emission is split into 8 thunks (silu chain / z chunk per
                i-tile pair) so evac ops of concurrent sweeps are not
                buried behind a whole quarter of z work in the engine
                queues."""

                def __init__(self, l, q, h_t):
                    self.l, self.q, self.h_t = l, q, h_t
                    self.st = None
                    self.pairs = []
                    self.remaining = 2 * KQ

                    def wrap(fn, ki):
                        def run():
                            fn(ki)
                            self.remaining -= 1
                        return run

                    for ki in range(KQ):
                        pending.append(wrap(self._silu, ki))
                        pending.append(wrap(self._chunk, ki))

                def ensure(self):
                    while self.remaining > 0 and pending:
                        pending.popleft()()

                def _silu(self, ki):
                    if self.st is None:
                        self.st = silup.tile([128, KQ, B], BF16, tag="silu",
                                             name=f"si{self.l}_{self.q}")
                    it = 4 * self.q + ki
                    h_t = self.h_t
                    # silu(h) = h * sigmoid(h), sigmoid via exp/ln1p/exp
                    e = rp.tile([128, B], F32, tag="r", name="sile")
                    nc.scalar.activation(e, h_t[:, it, :], AF.Exp, scale=-1.0)
                    nc.scalar.activation(e, e, AF.Ln, bias=1.0)
                    nc.scalar.activation(e, e, AF.Exp, scale=-1.0)
                    nc.vector.scalar_tensor_tensor(
                        self.st[:, ki, :], e, 0.0, h_t[:, it, :],
                        OP.add, OP.mult)

                def _chunk(self, ki):
                    if ki % 2 == 0:
                        self.pairs.append(basesp.tile(
                            [128, COEFF, 2, B], BF16, tag="bases",
                            name=f"ba{self.l}_{self.q}_{ki}"))
                    bt6 = self.pairs[ki // 2]
                    irel = ki % 2
                    hsl = self.h_t[:, 4 * self.q + ki, :]  # [128,512] contig
                    y = yp.tile([128, 10, B], F32, tag="y")
                    yf = y.rearrange("p m b -> p (m b)")
                    for m in range(10):
                        if m in SQ_SET:
                            # (h-g)^2 -> signed cube -> clamp to >=0
                            nc.scalar.activation(y[:, m], hsl, AF.Square,
                                                 bias=grid_sb[:, m:m + 1])
                            nc.vector.scalar_tensor_tensor(
                                y[:, m], hsl, grid_sb[:, m:m + 1],
                                y[:, m], OP.add, OP.mult)
                            nc.vector.tensor_scalar(y[:, m], y[:, m], 0.0,
                                                    None, OP.max)
                        else:
                            # relu^3 = exp(3*ln(relu(h-g)+1e-20))
                            r = rp.tile([128, B], F32, tag="r")
                            nc.scalar.activation(r, hsl, AF.Relu,
                                                 bias=grid_sb[:, m:m + 1])
                            nc.scalar.activation(r, r, AF.Ln,
                                                 bias=tiny[:, 0:1])
                            nc.scalar.activation(y[:, m], r, AF.Exp,
                                                 scale=3.0)
                    a = pp.tile([128, COEFF, B], F32, tag="a")
                    af = a.rearrange("p c b -> p (c b)")
                    nc.gpsimd.tensor_tensor(af, yf[:, 0:3072],
                                            yf[:, 2048:5120], OP.add)
                    nc.vector.scalar_tensor_tensor(af, yf[:, 512:3584], -4.0,
                                                   af, OP.mult, OP.add)
                    nc.vector.scalar_tensor_tensor(af, yf[:, 1536:4608], -4.0,
                                                   af, OP.mult, OP.add)
                    nc.vector.scalar_tensor_tensor(
                        bt6[:, :, irel, :], y[:, 2:8], 6.0, a,
                        OP.mult, OP.add)

            def z_quarter(l, q, h_t):
                return ZQ(l, q, h_t)

            def sweep(l, kt_lo, kt_hi, hacc, h2_t, zq, mode, post_blk,
                      drain_n):
                """matmuls over k-tiles [kt_lo, kt_hi) for all o of layer l.
                mode: 'copy' (PSUM->hacc), 'add' (hacc += PSUM), or
                'final' (h_next = hacc + PSUM; bf16 h2 when l==2)."""
                for q in set(kt // 4 for kt in range(kt_lo, kt_hi)):
                    zq[q].ensure()
                fo = dims[l + 1]
                nblk = fo // (OBLK * 128)
                for blk in range(nblk):
                    accs = [psA.tile([128, B], F32, tag="acc",
                                     name=f"a{l}_{kt_lo}_{blk}_{o}")
                            for o in range(OBLK)]
                    for kt in range(kt_lo, kt_hi):
                        wa = wkpa.tile([128, 4, OBLK * 128], BF16, tag="wka")
                        nc.sync.dma_start(
                            wa, wk[l][kt, blk, 0:4].rearrange("s p f -> p s f"))
                        wb = wkpb.tile([128, 3, OBLK * 128], BF16, tag="wkb")
                        nc.scalar.dma_start(
                            wb, wk[l][kt, blk, 4:7].rearrange("s p f -> p s f"))
                        z0 = zq[kt // 4]
                        for s in range(7):
                            wt_sl = wa[:, s, :] if s < 4 else wb[:, s - 4, :]
                            rhs = (z0.st[:, kt % 4, :] if s == 0
                                   else z0.pairs[(kt % 4) // 2][:, s - 1,
                                                                kt % 2, :])
                            for o in range(OBLK):
                                osl = slice(o * 128, (o + 1) * 128)
                                nc.tensor.matmul(
                                    accs[o], wt_sl[:, osl], rhs,
                                    start=(kt == kt_lo and s == 0),
                                    stop=(kt == kt_hi - 1 and s == 6))
                    for o in range(OBLK):
                        ot = OBLK * blk + o
                        if mode == "copy":
                            nc.scalar.copy(hacc[:, ot, :], accs[o])
                        elif mode == "final" and l == 2:
                            nc.vector.tensor_tensor(h2_t[:, ot, :], accs[o],
                                                    hacc[:, ot, :], OP.add)
                        else:
                            nc.vector.tensor_tensor(hacc[:, ot, :], accs[o],
                                                    hacc[:, ot, :], OP.add)
                    if post_blk is not None:
                        post_blk(blk)
                    drain(drain_n)

            # h2 lives in a bases-pool slot (allocated at layer-2 time)
            h2_holder = [None]

            # ---- layer flow ----
            # mult val blocks, with layer-0 z starting as soon as the h
            # slices each needs are final
            zq = {}
            for j in range(4):
                mult_block(4 + j, False, j)
                if j == 0:
                    zq[0] = z_quarter(0, 0, h_in)
                elif j == 1:
                    zq[1] = z_quarter(0, 1, h_in)
                drain(3 if j else 0)

            for l in range(3):
                nblk = dims[l + 1] // (OBLK * 128)
                dn = 8 // nblk
                hacc = hring.tile([128, 16, B], F16, tag="h", name=f"hacc{l}")
                zq[2] = z_quarter(l, 2, h_in)
                sweep(l, 0, 4, hacc, None, zq, "copy", None, dn + 1)
                zq[3] = z_quarter(l, 3, h_in)
                sweep(l, 4, 8, hacc, None, zq, "add", None, dn + 1)
                if l == 2:
                    bt = basesp.tile([128, COEFF, 2, B], BF16, tag="bases",
                                     name="h2t")
                    h2_holder[0] = bt.rearrange("p c i b -> p (c i) b")
                nxt_h = hacc
                nxt_zq = {}

                def post_blk(blk, l=l, nxt_h=nxt_h, nxt_zq=nxt_zq):
                    if l < 2 and blk <= 1:
                        nxt_zq[blk] = z_quarter(l + 1, blk, nxt_h)

                h2v = h2_holder[0]
                sweep(l, 8, 16, hacc, h2v, zq, "final", post_blk, dn + 1)
                h_in = hacc
                zq = nxt_zq

            # ---- heads ----
            h2_t = h2_holder[0]
            acc2 = psA.tile([128, B], F32, tag="acc", name="headacc")
            for kt in range(8):
                nc.tensor.matmul(acc2[0:2, :], hw_sb[:, kt, :],
                                 h2_t[:, kt, :], start=(kt == 0),
                                 stop=(kt == 7))
            res = consts.tile([2, B], F32, tag="res")
            nc.vector.tensor_scalar(res, acc2[0:2, :], hb_sb[:, 0:1], None,
                                    OP.add)
            nc.sync.dma_start(out[:], res)

    return _patch_json(nc)


_NC = None
_PACKED = None


def _pack_weights(inputs):
    import ml_dtypes
    bf16 = ml_dtypes.bfloat16
    dims = [D] + WIDTHS
    packed = {}
    # mult: wT[i, o] -> [16, 8, 128, 512]
    wT = np.ascontiguousarray(np.asarray(inputs["mult_w"], np.float32).T)
    packed["wm"] = np.ascontiguousarray(
        wT.reshape(16, 128, 8, 4, 128).transpose(0, 2, 1, 3, 4)
        .reshape(16, 8, 128, 512)).astype(bf16)
    packed["mb"] = np.ascontiguousarray(np.asarray(inputs["mult_b"], np.float32))
    for l in range(3):
        fi, fo = dims[l], dims[l + 1]
        nblk = fo // (OBLK * 128)
        bw = np.asarray(inputs[f"base_w{l}"], np.float32)
        sw = np.asarray(inputs[f"spline_w{l}"], np.float32)
        sc = np.asarray(inputs[f"scaler{l}"], np.float32)
        S = np.empty((fi, 7, fo), np.float32)
        S[:, 0, :] = bw.T
        S[:, 1:, :] = (sw * (sc[:, :, None] * LAM3)).transpose(1, 2, 0)
        packed[f"wk{l}"] = np.ascontiguousarray(
            S.reshape(16, 128, 7, nblk, OBLK * 128).transpose(0, 3, 2, 1, 4)
        ).astype(bf16)
    packed["hw"] = np.ascontiguousarray(np.stack(
        [np.asarray(inputs["reg_w"], np.float32)[0],
         np.asarray(inputs["aux_w"], np.float32)[0]], axis=1)).astype(bf16)
    packed["hb"] = np.array(
        [[float(np.asarray(inputs["reg_b"]).reshape(-1)[0])],
         [float(np.asarray(inputs["aux_b"]).reshape(-1)[0])]], np.float32)
    return packed


def kernel(**inputs):
    global _NC, _PACKED
    from concourse.bass_utils import run_bass_kernel_spmd

    if _NC is None:
        _NC = build()
    _PACKED = _pack_weights(inputs)
    x_full = np.ascontiguousarray(np.asarray(inputs["x"], np.float32))
    per_core = []
    for c in range(N_CORES):
        m = dict(_PACKED)
        m["x"] = np.ascontiguousarray(x_full[c * B:(c + 1) * B])
        per_core.append(m)
    res = run_bass_kernel_spmd(_NC, per_core, core_ids=list(range(N_CORES)))
    reg = np.concatenate([res.results[c]["out"][0] for c in range(N_CORES)])
    aux = np.concatenate([res.results[c]["out"][1] for c in range(N_CORES)])
    kernel.last_results = res
    return reg, aux
